# revision 28
# baseline (speedup 1.0000x reference)
"""Trainium2 Bass kernel for nn_CurvatureOnlyRegularizer (retrieval_knn).

Full inputs -> full output. Shards the 8192 points row-wise across 8 cores.

Per-core pipeline (1024 rows = 8 row-tiles of 128), single software-pipelined
loop (no halves):
  1. S* = e1 . e2^T via bf16 PE matmul (4 k-chunks of 128; no aug rows).
  2. ACT evacuates PSUM as t1 = Relu(psum*Q + bias_i), bias_i folds the
     1.5*2^23 magic constant + Q*(C0 - n_i): quantizes to integer q_i.
  3. One fused STT packs t2 = (t1 - MAGIC) + T[j] where the preloaded f32
     table T[j] = (j%1024)/1024 - round(Q*n_j) carries both the -n_j term
     (integer, so it can live outside the magic round) and the index frac.
     |m| < 2^14 keeps the 1/1024 frac exact in fp32.
  4. Per-1024-chunk max8 -> 64 candidates; top-16-of-64 + max_index recovers
     (chunk, frac) -> global idx and m -> d'^2.
  5. Neighbor embeddings via dma_gather(transpose=True); PE gram per point;
     single-hop 3D fold DMAs (16/tile) produce point-major ptR [128,256] f16.
  6. cos from gram + invd; ap_gather upper-triangle; max8/match_replace sort;
     MSE vs host-reversed references, accumulated on-chip.
  Phase F for tile t runs during tile t+2 (fold latency); gram(t-1) is
  emitted mid phase-A(t) so the PE never head-of-line blocks on the gather.
Host sums the 8 per-core partial sums.
"""

import os
from contextlib import ExitStack

import ml_dtypes
import numpy as np

import concourse.bass as bass
import concourse.bass_isa as bass_isa
import concourse.mybir as mybir
import concourse.tile as tile
from concourse import bacc
from concourse.bass import ds, ts
from concourse.bass_utils import run_bass_kernel_spmd

N, D, K = 8192, 512, 15
NCORES = 8
SHARD = N // NCORES            # 1024
RT = SHARD // 128              # 8 row-tiles per core
CHUNK = 1024
NCH = N // CHUNK               # 8 column chunks
MAGIC = 12582912.0             # 1.5 * 2^23
C0 = 2052.0
QSCALE = 12.5
UNPACK_OFF = -1023.0 / 2048.0
PAD_CURV = -1.0
PAD_ANG = -4.0
NEG_BIG = -3.0e38
F32 = mybir.dt.float32
F16 = mybir.dt.float16
BF16 = mybir.dt.bfloat16
I16 = mybir.dt.int16
U32 = mybir.dt.uint32
AX = mybir.AxisListType
OP = mybir.AluOpType
AF = mybir.ActivationFunctionType

# which 1024-chunks run their pack-STT on gpsimd (rest on vector).
# NOTE: TensorScalarPtr (STT) is not a legal Pool opcode on this codegen —
# keep empty; use plain tensor_tensor for any gpsimd offload instead.
STT_ON_GPSIMD = ()


def build_nc(debug_out: bool = False):
    nc = bacc.Bacc("TRN2", target_bir_lowering=False, debug=False)

    rhsT_d = nc.dram_tensor("rhsT", [512, N], BF16, kind="ExternalInput")
    lhsT_d = nc.dram_tensor("lhsT", [512, SHARD], BF16, kind="ExternalInput")
    egat_d = nc.dram_tensor("egather", [N, D], BF16, kind="ExternalInput")
    tpack_d = nc.dram_tensor("tpack", [128, N], F32, kind="ExternalInput")
    bias_d = nc.dram_tensor("bias", [128, RT], F32, kind="ExternalInput")
    self_d = nc.dram_tensor("selfidx", [128, RT], F32, kind="ExternalInput")
    refc_d = nc.dram_tensor("refc", [128, RT * 16], F32, kind="ExternalInput")
    refa_d = nc.dram_tensor("refa", [128, RT * 112], F32, kind="ExternalInput")
    part_d = nc.dram_tensor("partial", [1, 2], F32, kind="ExternalOutput")
    # DRAM bounce buffers for the gram fold (partition shuffle via flat DRAM
    # addressing: SBUF APs can't reorder partitions, DRAM APs can)
    fscr_d = nc.dram_tensor("foldscr", [2, 128, 256], F16, kind="Internal")
    if debug_out:
        dbg_idx_d = nc.dram_tensor("dbg_idx", [128, 16], F32, kind="ExternalOutput")
        dbg_d2_d = nc.dram_tensor("dbg_d2", [128, 16], F32, kind="ExternalOutput")
        dbg_srtc_d = nc.dram_tensor("dbg_srtc", [128, 16], F32, kind="ExternalOutput")
        dbg_ang_d = nc.dram_tensor("dbg_ang", [128, 112], F32, kind="ExternalOutput")
        dbg_cand_d = nc.dram_tensor("dbg_cand", [128, 64], F32, kind="ExternalOutput")
        dbg_ptr_d = nc.dram_tensor("dbg_ptr", [128, 256], F16, kind="ExternalOutput")
        dbg_gsk_d = nc.dram_tensor("dbg_gsk", [128, 2048], F16, kind="ExternalOutput")

    # preamble (before Tile body)
    r2048 = nc.gpsimd.to_reg(2048)
    rfill1 = nc.gpsimd.to_reg(1.0)

    with tile.TileContext(nc) as tc, ExitStack() as ctx:
        const = ctx.enter_context(tc.tile_pool(name="const", bufs=1))
        sel = ctx.enter_context(tc.tile_pool(name="sel", bufs=3))
        scr = ctx.enter_context(tc.tile_pool(name="scr", bufs=2))
        vbuf = ctx.enter_context(tc.tile_pool(name="vbuf", bufs=3))
        gskp = ctx.enter_context(tc.tile_pool(name="gskp", bufs=1))
        ptp = ctx.enter_context(tc.tile_pool(name="ptp", bufs=4))
        psS = ctx.enter_context(tc.tile_pool(name="psS", bufs=2, space="PSUM"))
        psG = ctx.enter_context(tc.tile_pool(name="psG", bufs=1, space="PSUM"))
        psT = ctx.enter_context(tc.tile_pool(name="psT", bufs=1, space="PSUM"))

        # ---- constants / resident data ----
        rhs_sb = [const.tile([128, N], BF16, tag=f"rhs{c}", name=f"rhs{c}") for c in range(4)]
        tpk_sb = const.tile([128, N], F32, tag="tpack")
        lhs_sb = [const.tile([128, SHARD], BF16, tag=f"lhs{c}", name=f"lhs{c}") for c in range(4)]
        bias_sb = const.tile([128, RT], F32, tag="bias")
        self_sb = const.tile([128, RT], F32, tag="self")
        refc_sb = const.tile([128, RT * 16], F32, tag="refc")
        refa_sb = const.tile([128, RT * 112], F32, tag="refa")
        perm = const.tile([128, 128], F32, tag="perm")
        repmat = const.tile([16, 128], F32, tag="repmat")
        invd_st = const.tile([128, RT * 16], F32, tag="invdst")
        css = const.tile([128, 1], F32, tag="css")
        ass = const.tile([128, 1], F32, tag="ass")

        # small consts first
        nc.sync.dma_start(bias_sb[:], bias_d.ap()[:])
        nc.sync.dma_start(self_sb[:], self_d.ap()[:])
        nc.scalar.dma_start(refc_sb[:], refc_d.ap()[:])
        nc.scalar.dma_start(refa_sb[:], refa_d.ap()[:])
        # big resident loads, column-halved so phase A can start early
        for half in range(2):
            cs = ds(half * 4096, 4096)
            for c in range(4):
                eng = nc.sync if c % 2 == 0 else nc.scalar
                eng.dma_start(rhs_sb[c][:, cs], rhsT_d.ap()[ts(c, 128), cs])
            nc.scalar.dma_start(tpk_sb[:, cs], tpack_d.ap()[:, cs])
            if half == 0:
                for c in range(4):
                    nc.sync.dma_start(lhs_sb[c][:], lhsT_d.ap()[ts(c, 128), :])

        # perm[k, 8u+v] = 1 iff k == 16v+u  (gram-position permutation)
        nc.gpsimd.memset(perm[:], 0.0)
        nc.gpsimd.affine_select(
            out=bass.AP(tensor=perm[:].tensor, offset=0,
                        ap=[perm[:].ap[0], [8, 16], [1, 8]]),
            in_=bass.AP(tensor=perm[:].tensor, offset=0,
                        ap=[perm[:].ap[0], [8, 16], [1, 8]]),
            compare_op=OP.not_equal,
            fill=rfill1, base=0,
            pattern=[[-1, 16], [-16, 8]],
            channel_multiplier=1,
        )
        # repmat[k, 16a+s] = 1 iff k == s  (partition-block replicator)
        nc.gpsimd.memset(repmat[:], 0.0)
        nc.gpsimd.affine_select(
            out=bass.AP(tensor=repmat[:].tensor, offset=0,
                        ap=[repmat[:].ap[0], [16, 8], [1, 16]]),
            in_=bass.AP(tensor=repmat[:].tensor, offset=0,
                        ap=[repmat[:].ap[0], [16, 8], [1, 16]]),
            compare_op=OP.not_equal,
            fill=rfill1, base=0,
            pattern=[[0, 8], [-1, 16]],
            channel_multiplier=1,
        )
        nc.vector.memset(css[:], 0.0)
        nc.vector.memset(ass[:], 0.0)

        # per-tile state carried across loop iterations
        state = {}        # t -> dict(vt=, ptR=, angv=, ...)

        def phase_A(t):
            """matmul + evac + pack + chunk-max8."""
            cand = sel.tile([128, 64], F32, tag="cand")
            for jg in range(NCH):
                pss = [
                    psS.tile([128, 512], F32, tag="psA", name="psA"),
                    psS.tile([128, 512], F32, tag="psB", name="psB"),
                ]
                t1 = scr.tile([128, 1024], F32, tag="t1")
                for q in range(2):
                    for k in range(4):
                        nc.tensor.matmul(
                            pss[q][:],
                            lhs_sb[k][:, ts(t, 128)],
                            rhs_sb[k][:, ds(jg * 1024 + q * 512, 512)],
                            start=(k == 0),
                            stop=(k == 3),
                        )
                    nc.scalar.activation(
                        t1[:, ts(q, 512)], pss[q][:], AF.Relu,
                        bias=bias_sb[:, t : t + 1], scale=QSCALE,
                    )
                t2 = scr.tile([128, 1024], F32, tag="t2")
                eng = nc.gpsimd if jg in STT_ON_GPSIMD else nc.vector
                eng.scalar_tensor_tensor(
                    t2[:], t1[:], -MAGIC, tpk_sb[:, ts(jg, 1024)],
                    op0=OP.add, op1=OP.add,
                )
                nc.vector.max(cand[:, ts(jg, 8)], t2[:])
            return cand

        def phase_BCD(t, cand):
            """select top-16, unpack, curvature, idx build + gather."""
            v16 = sel.tile([128, 16], F32, tag="v16")
            nc.vector.max(v16[:, 0:8], cand[:])
            candz = sel.tile([128, 64], F32, tag="candz")
            nc.vector.match_replace(candz[:], v16[:, 0:8], cand[:], NEG_BIG)
            nc.vector.max(v16[:, 8:16], candz[:])
            pos = sel.tile([128, 16], U32, tag="pos")
            nc.vector.max_index(pos[:, 0:8], v16[:, 0:8], cand[:])
            nc.vector.max_index(pos[:, 8:16], v16[:, 8:16], candz[:])
            chunk_u = sel.tile([128, 16], U32, tag="chunku")
            nc.vector.tensor_scalar(
                chunk_u[:], pos[:], 3, None, op0=OP.logical_shift_right
            )
            chunk_f = sel.tile([128, 16], F32, tag="chunkf")
            nc.vector.tensor_copy(chunk_f[:], chunk_u[:])
            # unpack m (integer part) via magic round
            s1 = sel.tile([128, 16], F32, tag="s1")
            nc.vector.tensor_scalar(s1[:], v16[:], UNPACK_OFF, None, op0=OP.add)
            wv = sel.tile([128, 16], F32, tag="wv")
            nc.scalar.activation(wv[:], s1[:], AF.Copy, bias=MAGIC, scale=1.0)
            m16 = sel.tile([128, 16], F32, tag="m16")
            nc.vector.tensor_scalar(m16[:], wv[:], -MAGIC, None, op0=OP.add)
            # frac = v16 - m16 = idx/1024 ; gidx = (chunk + frac) * 1024
            frac = sel.tile([128, 16], F32, tag="frac")
            nc.vector.scalar_tensor_tensor(
                frac[:], m16[:], -1.0, v16[:], op0=OP.mult, op1=OP.add
            )
            gidx = sel.tile([128, 16], F32, tag="gidx")
            nc.vector.tensor_tensor(gidx[:], chunk_f[:], frac[:], op=OP.add)
            nc.vector.tensor_scalar(gidx[:], gidx[:], float(CHUNK), None, op0=OP.mult)
            # dp2 = C0 - m/QSCALE (slot 0 = self, dropped)
            dp2 = sel.tile([128, 16], F32, tag="dp2")
            nc.vector.tensor_scalar(
                dp2[:], m16[:], -1.0 / QSCALE, C0, op0=OP.mult, op1=OP.add
            )
            # ---- curvature ----
            d2re = sel.tile([128, 16], F32, tag="d2re")
            nc.vector.tensor_scalar_max(d2re[:, 0:15], dp2[:, 1:16], 1e-12)
            nc.vector.memset(d2re[:, 15:16], 1.0)
            dt_ = sel.tile([128, 16], F32, tag="dt")
            nc.scalar.sqrt(dt_[:], d2re[:])
            nc.vector.reciprocal(invd_st[:, ts(t, 16)], dt_[:])
            dsum = sel.tile([128, 1], F32, tag="dsum")
            nc.vector.reduce_sum(dsum[:], dt_[:, 0:15], axis=AX.X)
            dmean = sel.tile([128, 1], F32, tag="dmean")
            nc.vector.tensor_scalar(
                dmean[:], dsum[:], 1.0 / 15.0, 1e-8, op0=OP.mult, op1=OP.add
            )
            ivm = sel.tile([128, 1], F32, tag="ivm")
            nc.vector.reciprocal(ivm[:], dmean[:])
            sig = sel.tile([128, 16], F32, tag="sig")
            nc.vector.tensor_scalar(
                sig[:, 0:15], dt_[:, 0:15], ivm[:], None, op0=OP.mult
            )
            nc.vector.memset(sig[:, 15:16], PAD_CURV)
            srtc = sel.tile([128, 16], F32, tag="srtc")
            nc.vector.max(srtc[:, 0:8], sig[:])
            sigz = sel.tile([128, 16], F32, tag="sigz")
            nc.vector.match_replace(sigz[:], srtc[:, 0:8], sig[:], -2.0)
            nc.vector.max(srtc[:, 8:16], sigz[:])
            dcv = sel.tile([128, 16], F32, tag="dcv")
            nc.vector.tensor_tensor(
                dcv[:], srtc[:], refc_sb[:, ts(t, 16)], op=OP.subtract
            )
            csq = sel.tile([128, 16], F32, tag="csq")
            css_t = sel.tile([128, 1], F32, tag="csst")
            nc.scalar.activation(csq[:], dcv[:], AF.Square, accum_out=css_t[:])
            nc.vector.tensor_tensor(css[:], css[:], css_t[:], op=OP.add)
            # ---- idx build + gather ----
            kif = sel.tile([128, 16], F32, tag="kif")
            nc.vector.tensor_copy(kif[:, 0:15], gidx[:, 1:16])
            nc.vector.tensor_copy(kif[:, 15:16], self_sb[:, t : t + 1])
            pst1 = psT.tile([16, 128], F32, tag="pst1", name="pst1")
            nc.tensor.transpose(pst1[:], kif[:], perm[:])
            t1s = sel.tile([16, 128], F32, tag="t1s")
            nc.vector.tensor_copy(t1s[:], pst1[:])
            pst = psT.tile([128, 128], F32, tag="pstT", name="pst")
            nc.tensor.matmul(pst[:], repmat[:], t1s[:], start=True, stop=True)
            idx16 = sel.tile([128, 128], I16, tag="idx16")
            nc.vector.tensor_copy(idx16[:], pst[:])
            vt = vbuf.tile([128, 4, 2048], BF16, tag="vt")
            nc.gpsimd.dma_gather(
                out_ap=vt[:],
                in_ap=egat_d.ap()[:],
                idxs_ap=idx16[:],
                num_idxs=2048,
                num_idxs_reg=r2048,
                elem_size=512,
                transpose=True,
                single_packet=False,
            )
            st = {"vt": vt, "kif": kif, "dp2": dp2, "srtc": srtc, "cand": cand}
            return st

        def gram_fold(t):
            """gram matmuls + evac + DRAM-bounce fold -> ptR(t) [128,256]."""
            vt = state[t]["vt"]
            gsk = gskp.tile([128, 2048], F16, tag="gsk", name="gsk")
            for w in range(2):
                pg = psG.tile([128, 1024], F32, tag="pg", name="pg")
                for g2h in range(8):
                    g2 = 8 * w + g2h
                    for c in range(4):
                        nc.tensor.matmul(
                            pg[:, ts(g2h, 128)],
                            vt[:, c, ts(g2, 128)],
                            vt[:, c, ts(g2, 128)],
                            start=(c == 0),
                            stop=(c == 3),
                        )
                nc.scalar.activation(gsk[:, ts(w, 1024)], pg[:], AF.Copy)
            # hop-1 to DRAM, point-major: element (p; l, g2', m) ->
            # fscr[b, 16*p+g2', l*16+m]   (g2' = 8w+g2, src col = 128*g2'+16p+m;
            # the idx-build perm maps gather block P=8u+v to point 16v+u, so
            # group p's 16 blocks are points 16p..16p+15)
            b = t % 2
            fap = fscr_d.ap()
            for p in range(8):
                src = bass.AP(
                    tensor=gsk[:].tensor,
                    offset=gsk[:].offset + 16 * p * 2048 + 16 * p,
                    ap=[[2048, 16], [128, 16], [1, 16]],
                )
                dst = bass.AP(
                    tensor=fap.tensor,
                    offset=fap.offset + (b * 128 + 16 * p) * 256,
                    ap=[[16, 16], [256, 16], [1, 16]],
                )
                eng = nc.sync if p % 2 == 0 else nc.scalar
                eng.dma_start(dst, src)
            # hop-2: contiguous readback into point-major SBUF tile
            ptR = ptp.tile([128, 256], F16, tag="ptR")
            nc.sync.dma_start(ptR[:], fscr_d.ap()[b])
            state[t]["ptR"] = ptR
            if debug_out and t == 0:
                nc.sync.dma_start(dbg_ptr_d.ap()[:], ptR[:])
                nc.sync.dma_start(dbg_gsk_d.ap()[:], gsk[:])

        def phase_F_cos(t):
            """raw -> cos + triu gather (emitted early in t+2)."""
            ptR = state[t]["ptR"]
            prt = ptR[:]
            p0 = prt.ap[0]
            base = prt.offset
            pr = bass.AP(tensor=prt.tensor, offset=base,
                         ap=[p0, [16, 16], [1, 16]])
            in_l15 = bass.AP(tensor=prt.tensor, offset=base + 15,
                             ap=[p0, [16, 16], [0, 16]])
            in_r15 = bass.AP(tensor=prt.tensor, offset=base + 15 * 16,
                             ap=[p0, [0, 16], [1, 16]])
            ta = sel.tile([128, 256], F32, tag="ta")
            nc.vector.tensor_tensor(ta[:], pr, in_l15, op=OP.subtract)
            tb = sel.tile([128, 256], F32, tag="tb")
            nc.vector.tensor_tensor(tb[:], ta[:], in_r15, op=OP.subtract)
            ivt = invd_st[:, ts(t, 16)]
            iv_l = bass.AP(
                tensor=ivt.tensor, offset=ivt.offset,
                ap=[ivt.ap[0], [1, 16], [0, 16]],
            )
            iv_m = bass.AP(
                tensor=ivt.tensor, offset=ivt.offset,
                ap=[ivt.ap[0], [0, 16], [1, 16]],
            )
            r1515 = bass.AP(
                tensor=prt.tensor, offset=base + 15 * 16 + 15,
                ap=[p0, [1, 1]],
            )
            tcc = sel.tile([128, 256], F32, tag="tc")
            nc.vector.scalar_tensor_tensor(
                tcc[:], tb[:], r1515, iv_l, op0=OP.add, op1=OP.mult
            )
            cosv = sel.tile([128, 256], F32, tag="cosv")
            nc.vector.tensor_tensor(cosv[:], tcc[:], iv_m, op=OP.mult)
            # upper-triangle extraction via 14 strided copies (keeps the
            # GpSimd engine gather-only: avoids ~6us ucode library reloads
            # on every APGather<->DMAGatherAnt alternation)
            angv = sel.tile([128, 112], F32, tag="angv")
            off = 0
            for l in range(14):
                ln = 14 - l
                nc.vector.tensor_copy(
                    angv[:, off : off + ln],
                    cosv[:, l * 16 + l + 1 : l * 16 + 15],
                )
                off += ln
            nc.vector.memset(angv[:, 105:112], PAD_ANG)
            state[t]["angv"] = angv

        def phase_F_sort(t):
            """sort angv desc + MSE accumulate (emitted late in t+2)."""
            angv = state[t]["angv"]
            srta = sel.tile([128, 112], F32, tag="srta")
            work = angv
            for r in range(14):
                nc.vector.max(srta[:, ts(r, 8)], work[:])
                if r < 13:
                    nwork = sel.tile([128, 112], F32, tag=f"work{r % 2}")
                    nc.vector.match_replace(
                        nwork[:], srta[:, ts(r, 8)], work[:], NEG_BIG
                    )
                    work = nwork
            dav = sel.tile([128, 112], F32, tag="dav")
            nc.vector.tensor_tensor(
                dav[:], srta[:], refa_sb[:, ts(t, 112)], op=OP.subtract
            )
            asq = sel.tile([128, 112], F32, tag="asq")
            ass_t = sel.tile([128, 1], F32, tag="asst")
            nc.scalar.activation(asq[:], dav[:], AF.Square, accum_out=ass_t[:])
            nc.vector.tensor_tensor(ass[:], ass[:], ass_t[:], op=OP.add)
            if debug_out and t == 0:
                nc.sync.dma_start(dbg_ang_d.ap()[:], srta[:])

        # Schedule: gram(t-2) at the END of tile t (its gather is 2 periods
        # old -> never blocks the PE queue); phase F for tile t-4 inside
        # tile t (ptR(t-4) landed mid tile t-1 -> never stalls the DVE).
        for t in range(RT):
            if t >= 4:
                phase_F_cos(t - 4)
            cand = phase_A(t)
            st = phase_BCD(t, cand)
            state[t] = {**st}
            if t >= 4:
                phase_F_sort(t - 4)
                del state[t - 4]
            if t >= 2:
                gram_fold(t - 2)
            if debug_out and t == 0:
                nc.sync.dma_start(dbg_idx_d.ap()[:], st["kif"][:])
                nc.sync.dma_start(dbg_d2_d.ap()[:], st["dp2"][:])
                nc.sync.dma_start(dbg_srtc_d.ap()[:], st["srtc"][:])
                nc.sync.dma_start(dbg_cand_d.ap()[:], st["cand"][:])

        # drain: gram(6), gram(7), F(4..7)
        gram_fold(RT - 2)
        phase_F_cos(RT - 4)
        phase_F_sort(RT - 4)
        gram_fold(RT - 1)
        for tt in range(RT - 3, RT):
            phase_F_cos(tt)
            phase_F_sort(tt)

        # ---- final reduce + output ----
        cred = const.tile([128, 1], F32, tag="cred")
        ared = const.tile([128, 1], F32, tag="ared")
        nc.gpsimd.partition_all_reduce(
            cred[:], css[:], channels=128, reduce_op=bass_isa.ReduceOp.add
        )
        nc.gpsimd.partition_all_reduce(
            ared[:], ass[:], channels=128, reduce_op=bass_isa.ReduceOp.add
        )
        fin = const.tile([1, 2], F32, tag="fin")
        nc.vector.tensor_copy(fin[0:1, 0:1], cred[0:1, :])
        nc.vector.tensor_copy(fin[0:1, 1:2], ared[0:1, :])
        nc.sync.dma_start(part_d.ap()[:], fin[:])

    nc.compile()
    return nc


# =====================  host side  =====================

def _prep_inputs(embeddings, reference_curvature_sig, reference_angular_sig):
    emb32 = np.asarray(embeddings, dtype=np.float32)
    refc = np.asarray(reference_curvature_sig, dtype=np.float32)
    refa = np.asarray(reference_angular_sig, dtype=np.float32)

    e1_bf = (np.float32(np.sqrt(2.0)) * emb32).astype(ml_dtypes.bfloat16)
    e1 = e1_bf.astype(np.float32)
    e2_bf = (2.0 * e1).astype(ml_dtypes.bfloat16)       # exact x2
    n1 = np.sum(e1.astype(np.float64) * e1.astype(np.float64), axis=1).astype(
        np.float32
    )

    rhsT = np.ascontiguousarray(e2_bf.T)                # [512, N]
    lhsT_full = np.ascontiguousarray(e1_bf.T)           # [512, N]

    Rj = np.rint(np.float64(QSCALE) * n1.astype(np.float64)).astype(np.float32)
    ks = (np.arange(N, dtype=np.float64) % CHUNK) / CHUNK
    tpack = (ks - Rj.astype(np.float64)).astype(np.float32)[None, :].repeat(
        128, axis=0
    )

    shared = dict(rhsT=rhsT, egather=e1_bf, tpack=tpack)
    per_core = []
    for c in range(NCORES):
        lo = c * SHARD
        sl = slice(lo, lo + SHARD)
        bias = (MAGIC + (C0 - n1[sl].astype(np.float64)) * QSCALE).astype(
            np.float32
        )
        bias_t = bias.reshape(RT, 128).T.copy()         # [128, RT]
        selfidx = (np.arange(lo, lo + SHARD, dtype=np.float32)
                   .reshape(RT, 128).T.copy())
        refc_c = np.full((SHARD, 16), PAD_CURV, dtype=np.float32)
        refc_c[:, 0:15] = refc[sl, ::-1]
        refc_t = (refc_c.reshape(RT, 128, 16).transpose(1, 0, 2)
                  .reshape(128, RT * 16).copy())
        refa_c = np.full((SHARD, 112), PAD_ANG, dtype=np.float32)
        refa_c[:, 0:105] = refa[sl, ::-1]
        refa_t = (refa_c.reshape(RT, 128, 112).transpose(1, 0, 2)
                  .reshape(128, RT * 112).copy())
        per_core.append(dict(
            shared,
            lhsT=np.ascontiguousarray(lhsT_full[:, sl]),
            bias=bias_t,
            selfidx=selfidx,
            refc=refc_t,
            refa=refa_t,
        ))
    return per_core


_NC_CACHE = {}


def run_cores(inputs, debug_out=False, **run_kwargs):
    key = debug_out
    if key not in _NC_CACHE:
        _NC_CACHE[key] = build_nc(debug_out=debug_out)
    nc = _NC_CACHE[key]
    in_maps = _prep_inputs(**inputs)
    res = run_bass_kernel_spmd(
        nc, in_maps, core_ids=list(range(NCORES)), **run_kwargs
    )
    return res


def kernel(embeddings, reference_curvature_sig, reference_angular_sig):
    res = run_cores(dict(
        embeddings=embeddings,
        reference_curvature_sig=reference_curvature_sig,
        reference_angular_sig=reference_angular_sig,
    ))
    css = 0.0
    ass = 0.0
    for r in res.results:
        css += float(r["partial"][0, 0])
        ass += float(r["partial"][0, 1])
    curv_loss = css / (N * 15)
    ang_loss = ass / (N * 105)
    out = np.float32(0.3 * curv_loss + 0.7 * ang_loss)
    return np.asarray(out, dtype=np.float32)
